# revision 1
# baseline (speedup 1.0000x reference)
"""Trainium2 Bass kernel for CrossLoRALinear:
    y = x @ W_base^T + b_base + ((x @ A^T) @ B^T) * SCALE

Strategy (8 NeuronCores, SPMD):
  - Data-parallel shard of the 4*4096=16384 tokens: 2048 tokens/core.
  - Replicate W_base/A/B/b_base.
  - Host casts x and W_baseT to bf16 (output error ~1e-3 vs the 2e-2 gate):
    halves HBM traffic and SBUF footprint; PE rate is the same as f32r.
  - On device, fold the rank-16 LoRA update into the weights once:
        W_effT[k,o] = round_bf16(W_baseT[k,o] + SCALE * (A^T @ B^T)[k,o])
    (64 K=16 matmuls + DVE adds), keeping W_effT resident in SBUF.
  - bias is broadcast across partitions with a K=1 PE matmul against a
    ones row (avoids a 128-descriptor broadcast DMA on the critical
    prologue stream).
  - Main GEMM per core: out[t,o] = sum_k xT[k,t]^T @ W_effT[k,o], fp32
    PSUM accumulation, bias fused into the PSUM->SBUF eviction (DVE).
  - Warm-start groups overlap the main GEMM with the W-load stream.
  - Host does layout only: shard/transpose/cast x, transpose W/B, concat.
"""
import sys

if "/opt/trn_rl_repo" not in sys.path:
    sys.path.insert(0, "/opt/trn_rl_repo")

import numpy as np
import ml_dtypes

N_CORES = 8
B_, S, D_IN, D_OUT, R = 4, 4096, 2048, 2048, 16
SCALE = 1.0
T_CORE = (B_ * S) // N_CORES  # 2048 tokens per core
P = 128
N_TT = T_CORE // P   # 16 token tiles per core
N_KT = D_IN // P     # 16 contraction tiles
OC_W = 512           # psum bank width (fp32)
N_OC = D_OUT // OC_W # 4 output chunks

_CACHE = {}


def _build_program(
    n_iters: int = 1,
    internal_io: bool = False,
    scope: str = "full",
):
    import concourse.bacc as bacc
    import concourse.mybir as mybir
    import concourse.bass as bass
    from concourse import tile

    dt = mybir.dt
    nc = bacc.Bacc(None, target_bir_lowering=False, debug=False)

    if internal_io:
        # Timing-only variant: big tensors live in device DRAM (contents
        # irrelevant for dense GEMM timing); tiny external I/O keeps the
        # graph alive and dispatch overhead minimal.
        x_in = nc.dram_tensor("x4_d", [N_TT, P, N_KT, P], dt.bfloat16)[:]
        w_in = nc.dram_tensor("wT_d", [D_IN, D_OUT], dt.bfloat16)[:]
        ab_in = nc.dram_tensor("ab_d", [R, D_IN + D_OUT], dt.float32r)[:]
        bias_in = nc.dram_tensor("bias_d", [D_OUT], dt.float32)[:]
        out_ext = nc.dram_tensor("out_d", [T_CORE, D_OUT], dt.float32)[:]
        dummy_in = nc.declare_dram_parameter("tick", [P, 4], dt.float32, isOutput=False)
        dummy_out = nc.declare_dram_parameter("tock", [P, 4], dt.float32, isOutput=True)
    else:
        # x4/wT are bf16 (host-cast). ab is A and B^T concatenated on the
        # free axis so one DMA loads both; declared float32r so the raw fp32
        # bits land ready for full-rate f32r fold matmuls (no cast pass).
        x_in = nc.declare_dram_parameter(
            "x4", [N_TT, P, N_KT, P], dt.bfloat16, isOutput=False
        )
        w_in = nc.declare_dram_parameter(
            "wT", [D_IN, D_OUT], dt.bfloat16, isOutput=False
        )
        ab_in = nc.declare_dram_parameter(
            "ab", [R, D_IN + D_OUT], dt.float32r, isOutput=False
        )
        bias_in = nc.declare_dram_parameter("bias", [D_OUT], dt.float32, isOutput=False)
        out_ext = nc.declare_dram_parameter(
            "out", [T_CORE, D_OUT], dt.float32, isOutput=True
        )

    # Warm-start groups: (tt, oc) psum groups opened during the W load so the
    # PE consumes each W k-tile as it arrives instead of idling behind the
    # weight DMA stream (PE executes in emission order).
    WARM = ((0, 0), (0, 1), (0, 2), (0, 3), (1, 0), (1, 1))

    def prologue(tc, pools, warm=False):
        const, wpool, wstage, xpool, opool, psumF, psumM = pools
        # --- A | B^T in one DMA (f32r bits straight from DRAM) ---
        ab_sb = const.tile([R, D_IN + D_OUT], dt.float32r, tag="ab_sb")
        nc.scalar.dma_start(out=ab_sb[:], in_=ab_in[:])
        a_sb = ab_sb[:, :D_IN]
        bt_sb = ab_sb[:, D_IN:]

        # bias row: one 8KB descriptor into partition 0
        bias_row = const.tile([1, D_OUT], dt.float32, tag="bias_row")
        bias_ap = bias_in[:]
        bias_row_src = bass.AP(
            tensor=bias_ap.tensor,
            offset=bias_ap.offset,
            ap=[[0, 1]] + list(bias_ap.ap),
        )
        nc.scalar.dma_start(out=bias_row[:], in_=bias_row_src)
        ones = const.tile([1, P], dt.float32, tag="ones")
        nc.vector.memset(ones[:], 1.0)
        bias_sb = const.tile([P, D_OUT], dt.float32, tag="bias_sb")

        if warm:
            warm_psums = {
                (tt, oc): psumM.tile(
                    [P, OC_W], dt.float32, tag="psM", name=f"wps_{tt}_{oc}"
                )
                for (tt, oc) in WARM
            }
        else:
            xts, warm_psums = None, None

        # --- W_effT: HWDGE-load W_baseT bf16, fold LoRA + round via DVE,
        # and feed the warm-start groups — all interleaved per k-tile so the
        # PE tracks the W DMA stream. k-tile 0 is DMA'd in OC_W-wide chunks
        # so the first fold matmul starts early; the warm x tiles ride the
        # ACT ring in parallel.
        w_tiles = []
        for kt in range(N_KT):
            ws = wstage.tile([P, D_OUT], dt.bfloat16, tag="ws")
            nc.sync.dma_start(out=ws[:], in_=w_in[kt * P : (kt + 1) * P, :])
            wt = wpool.tile([P, D_OUT], dt.bfloat16, tag=f"w{kt}")
            w_tiles.append(wt)
            for oc in range(N_OC):
                ps = psumF.tile([P, OC_W], dt.float32, tag="psF")
                nc.tensor.matmul(
                    ps[:],
                    a_sb[:, kt * P : (kt + 1) * P],
                    bt_sb[:, oc * OC_W : (oc + 1) * OC_W],
                    start=True,
                    stop=True,
                )
                # W_eff = round_bf16(W_base + SCALE * (BA)^T); SCALE == 1.0
                nc.vector.tensor_add(
                    out=wt[:, oc * OC_W : (oc + 1) * OC_W],
                    in0=ps[:],
                    in1=ws[:, oc * OC_W : (oc + 1) * OC_W],
                )
            if kt == 0:
                if warm:
                    # early x tiles for warm-start groups (ACT ring — off
                    # the W ring, queued behind W k-tile 0 only)
                    xt0 = xpool.tile([P, N_KT, P], dt.bfloat16, tag="xt")
                    nc.scalar.dma_start(out=xt0[:], in_=x_in[0])
                    xt1 = xpool.tile([P, N_KT, P], dt.bfloat16, tag="xt")
                    nc.scalar.dma_start(out=xt1[:], in_=x_in[1])
                    xts = (xt0, xt1)
                # broadcast bias across partitions: ones[1,P].T @ bias_row
                for oc in range(N_OC):
                    bps = psumF.tile([P, OC_W], dt.float32, tag="psF")
                    nc.tensor.matmul(
                        bps[:],
                        ones[:],
                        bias_row[:, oc * OC_W : (oc + 1) * OC_W],
                        start=True,
                        stop=True,
                    )
                    nc.vector.tensor_copy(
                        bias_sb[:, oc * OC_W : (oc + 1) * OC_W], bps[:]
                    )
            if warm:
                for (tt, oc) in WARM:
                    nc.tensor.matmul(
                        warm_psums[(tt, oc)][:],
                        xts[tt][:, kt, :],
                        wt[:, oc * OC_W : (oc + 1) * OC_W],
                        start=(kt == 0),
                        stop=(kt == N_KT - 1),
                    )
        return w_tiles, bias_sb, xts, warm_psums

    def evict(tt, oc, ps, opool, bias_sb, include_outdma):
        ot = opool.tile([P, OC_W], dt.float32, tag="ot")
        nc.vector.tensor_add(
            out=ot[:],
            in0=ps[:],
            in1=bias_sb[:, oc * OC_W : (oc + 1) * OC_W],
        )
        if include_outdma or tt == 0:
            # ACT-ring HWDGE: keeps stores off the SP ring
            nc.scalar.dma_start(
                out=out_ext[tt * P : (tt + 1) * P, oc * OC_W : (oc + 1) * OC_W],
                in_=ot[:],
            )

    def main_loop(
        tc,
        pools,
        w_tiles,
        bias_sb,
        xts=None,
        warm_psums=None,
        include_xdma=True,
        include_outdma=True,
    ):
        const, wpool, wstage, xpool, opool, psumF, psumM = pools
        warm = warm_psums is not None
        if warm:
            for (tt, oc) in WARM:
                evict(tt, oc, warm_psums[(tt, oc)], opool, bias_sb, include_outdma)
        # --- main GEMM over token tiles ---
        xt_fixed = None
        for tt in range(N_TT):
            if warm and tt < len(xts):
                xt = xts[tt]
            elif include_xdma or tt == 0:
                # plain HWDGE: sync ring is idle once W is in
                xt = xpool.tile([P, N_KT, P], dt.bfloat16, tag="xt")
                nc.sync.dma_start(out=xt[:], in_=x_in[tt])
                xt_fixed = xt
            else:
                xt = xt_fixed
            for oc in range(N_OC):
                if warm and (tt, oc) in WARM:
                    continue
                ps = psumM.tile([P, OC_W], dt.float32, tag="psM")
                for kt in range(N_KT):
                    nc.tensor.matmul(
                        ps[:],
                        xt[:, kt, :],
                        w_tiles[kt][:, oc * OC_W : (oc + 1) * OC_W],
                        start=(kt == 0),
                        stop=(kt == N_KT - 1),
                    )
                evict(tt, oc, ps, opool, bias_sb, include_outdma)

    with tile.TileContext(nc) as tc:
        with (
            tc.tile_pool(name="const", bufs=1) as const,
            tc.tile_pool(name="wpool", bufs=1) as wpool,
            tc.tile_pool(name="wstage", bufs=6) as wstage,
            tc.tile_pool(name="xpool", bufs=3) as xpool,
            tc.tile_pool(name="opool", bufs=3) as opool,
            tc.tile_pool(name="psumF", bufs=2, space="PSUM") as psumF,
            tc.tile_pool(name="psumM", bufs=6, space="PSUM") as psumM,
        ):
            pools = (const, wpool, wstage, xpool, opool, psumF, psumM)
            if n_iters == 1:
                assert scope == "full"
                w_tiles, bias_sb, xts, wp = prologue(tc, pools, warm=True)
                main_loop(tc, pools, w_tiles, bias_sb, xts, wp)
            elif scope == "full":
                with tc.For_i(0, n_iters, 1):
                    w_tiles, bias_sb, xts, wp = prologue(tc, pools, warm=True)
                    main_loop(tc, pools, w_tiles, bias_sb, xts, wp)
            else:
                w_tiles, bias_sb, _, _ = prologue(tc, pools, warm=False)
                with tc.For_i(0, n_iters, 1):
                    main_loop(
                        tc,
                        pools,
                        w_tiles,
                        bias_sb,
                        include_xdma=(scope != "pe"),
                        include_outdma=(scope != "pe"),
                    )
            if internal_io:
                tk = const.tile([P, 4], dt.float32, tag="tick")
                nc.sync.dma_start(out=tk[:], in_=dummy_in[:])
                nc.sync.dma_start(out=dummy_out[:], in_=tk[:])
    nc.compile()
    return nc


class _SpmdRunner:
    """Mirrors concourse.bass2jax.run_bass_via_pjrt but keeps the jitted
    executable alive so repeated calls don't recompile."""

    def __init__(self, nc, n_cores: int):
        import jax
        from jax.sharding import Mesh, PartitionSpec
        from jax.experimental.shard_map import shard_map
        import concourse.mybir as mybir
        from concourse.bass2jax import (
            _bass_exec_p,
            install_neuronx_cc_hook,
            partition_id_tensor,
        )

        install_neuronx_cc_hook()
        self.nc = nc
        self.n_cores = n_cores
        partition_name = (
            nc.partition_id_tensor.name if nc.partition_id_tensor else None
        )
        in_names, out_names, out_avals, zero_shapes = [], [], [], []
        for alloc in nc.m.functions[0].allocations:
            if not isinstance(alloc, mybir.MemoryLocationSet):
                continue
            name = alloc.memorylocations[0].name
            if alloc.kind == "ExternalInput":
                if name != partition_name:
                    in_names.append(name)
            elif alloc.kind == "ExternalOutput":
                shape = tuple(alloc.tensor_shape)
                dtype = mybir.dt.np(alloc.dtype)
                out_names.append(name)
                out_avals.append(jax.core.ShapedArray(shape, dtype))
                zero_shapes.append((shape, dtype))
        self.in_param_names = list(in_names)
        self.out_names = out_names
        self.out_avals = tuple(out_avals)
        self.zero_shapes = zero_shapes
        n_params = len(in_names)
        all_in_names = in_names + out_names
        if partition_name is not None:
            all_in_names.append(partition_name)
        n_outs = len(out_names)
        donate = tuple(range(n_params, n_params + n_outs))

        def _body(*args):
            operands = list(args)
            if partition_name is not None:
                operands.append(partition_id_tensor())
            outs = _bass_exec_p.bind(
                *operands,
                out_avals=self.out_avals,
                in_names=tuple(all_in_names),
                out_names=tuple(out_names),
                lowering_input_output_aliases=(),
                sim_require_finite=True,
                sim_require_nnan=True,
                nc=nc,
            )
            return tuple(outs)

        devices = jax.devices()[:n_cores]
        assert len(devices) == n_cores, (
            f"need {n_cores} neuron cores, found {len(jax.devices())}"
        )
        mesh = Mesh(np.asarray(devices), ("core",))
        in_specs = (PartitionSpec("core"),) * (n_params + n_outs)
        out_specs = (PartitionSpec("core"),) * n_outs
        self.sharded = jax.jit(
            shard_map(
                _body,
                mesh=mesh,
                in_specs=in_specs,
                out_specs=out_specs,
                check_rep=False,
            ),
            donate_argnums=donate,
            keep_unused=True,
        )

    def concat_inputs(self, in_maps):
        return [
            np.concatenate(
                [np.asarray(in_maps[c][n]) for c in range(self.n_cores)], axis=0
            )
            for n in self.in_param_names
        ]

    def _zeros(self):
        return [
            np.zeros((self.n_cores * s[0], *s[1:]), d)
            for (s, d) in self.zero_shapes
        ]

    def run_concat(self, concat_in):
        return self.sharded(*concat_in, *self._zeros())

    def run(self, in_maps):
        out_arrs = self.run_concat(self.concat_inputs(in_maps))
        res = []
        for c in range(self.n_cores):
            m = {}
            for i, name in enumerate(self.out_names):
                s = self.out_avals[i].shape
                m[name] = np.asarray(out_arrs[i]).reshape(self.n_cores, *s)[c]
            res.append(m)
        return res


def get_runner(n_iters: int = 1):
    key = ("runner", n_iters)
    if key not in _CACHE:
        nc = _build_program(n_iters=n_iters)
        _CACHE[key] = _SpmdRunner(nc, N_CORES)
    return _CACHE[key]


def make_in_maps(x, W_base, b_base, A, B):
    bf16 = ml_dtypes.bfloat16
    x2d = np.ascontiguousarray(x, dtype=np.float32).reshape(B_ * S, D_IN)
    wT = np.ascontiguousarray(W_base.T.astype(bf16))
    ab = np.ascontiguousarray(
        np.concatenate([A, B.T], axis=1), dtype=np.float32
    )
    bias = np.ascontiguousarray(b_base, dtype=np.float32)
    in_maps = []
    for c in range(N_CORES):
        xc = x2d[c * T_CORE : (c + 1) * T_CORE]  # [2048 t, 2048 k]
        # x4[tt, p(k), kt, t] = xc[tt*128 + t, kt*128 + p]  (SBUF layout)
        x4 = np.ascontiguousarray(
            xc.reshape(N_TT, P, N_KT, P).transpose(0, 3, 2, 1).astype(bf16)
        )
        in_maps.append({"x4": x4, "wT": wT, "ab": ab, "bias": bias})
    return in_maps


def kernel(**inputs):
    x = inputs["x"]
    W_base = inputs["W_base"]
    b_base = inputs["b_base"]
    A = inputs["A"]
    B = inputs["B"]
    runner = get_runner()
    in_maps = make_in_maps(x, W_base, b_base, A, B)
    res = runner.run(in_maps)
    y2d = np.concatenate([res[c]["out"] for c in range(N_CORES)], axis=0)
    return np.ascontiguousarray(y2d.reshape(B_, S, D_OUT), dtype=np.float32)



# revision 5
# speedup vs baseline: 1.0065x; 1.0065x over previous
"""Trainium2 Bass kernel for CrossLoRALinear:
    y = x @ W_base^T + b_base + ((x @ A^T) @ B^T) * SCALE

Strategy (8 NeuronCores, SPMD):
  - Data-parallel shard of the 4*4096=16384 tokens: 2048 tokens/core.
  - Replicate W_base/A/B/b_base.
  - Host casts x and W_baseT to bf16 (output error ~1e-3 vs the 2e-2 gate):
    halves HBM traffic and SBUF footprint; PE rate is the same as f32r.
  - On device, fold the rank-16 LoRA update into the weights once:
        W_effT[k,o] = round_bf16(W_baseT[k,o] + SCALE * (A^T @ B^T)[k,o])
    (64 K=16 matmuls + DVE adds), keeping W_effT resident in SBUF.
  - bias is broadcast across partitions with a K=1 PE matmul against a
    ones row (avoids a 128-descriptor broadcast DMA on the critical
    prologue stream).
  - Main GEMM per core: out[t,o] = sum_k xT[k,t]^T @ W_effT[k,o], fp32
    PSUM accumulation, bias fused into the PSUM->SBUF eviction (DVE).
  - Stationary-operand reuse: the token tile x[k,t] is the PE-stationary
    operand; the loop is kt-outer / oc-inner so ONE weight load feeds the
    4 output-chunk matmuls of that k-tile (4 MMs per LDWEIGHTS instead of
    1), keeping the PE at its N=512 streaming rate.
  - Warm-start groups overlap the main GEMM with the W-load stream.
  - Output stores alternate between the two HWDGE rings.
  - Host does layout only: shard/transpose/cast x, transpose W/B, concat.
"""
import sys

if "/opt/trn_rl_repo" not in sys.path:
    sys.path.insert(0, "/opt/trn_rl_repo")

import numpy as np
import ml_dtypes

N_CORES = 8
B_, S, D_IN, D_OUT, R = 4, 4096, 2048, 2048, 16
SCALE = 1.0
T_CORE = (B_ * S) // N_CORES  # 2048 tokens per core
P = 128
N_TT = T_CORE // P   # 16 token tiles per core
N_KT = D_IN // P     # 16 contraction tiles
OC_W = 512           # psum bank width (fp32)
N_OC = D_OUT // OC_W # 4 output chunks

_CACHE = {}


def _build_program(
    n_iters: int = 1,
    internal_io: bool = False,
    scope: str = "full",
):
    import concourse.bacc as bacc
    import concourse.mybir as mybir
    import concourse.bass as bass
    from concourse import tile

    dt = mybir.dt
    nc = bacc.Bacc(None, target_bir_lowering=False, debug=False)

    if internal_io:
        # Timing-only variant: big tensors live in device DRAM (contents
        # irrelevant for dense GEMM timing); tiny external I/O keeps the
        # graph alive and dispatch overhead minimal.
        x_in = nc.dram_tensor("x4_d", [N_TT, P, N_KT, P], dt.bfloat16)[:]
        w_in = nc.dram_tensor("wT_d", [D_IN, D_OUT], dt.bfloat16)[:]
        ab_in = nc.dram_tensor("ab_d", [R, D_IN + D_OUT], dt.float32r)[:]
        bias_in = nc.dram_tensor("bias_d", [D_OUT], dt.float32)[:]
        out_ext = nc.dram_tensor("out_d", [T_CORE, D_OUT], dt.float32)[:]
        dummy_in = nc.declare_dram_parameter("tick", [P, 4], dt.float32, isOutput=False)
        dummy_out = nc.declare_dram_parameter("tock", [P, 4], dt.float32, isOutput=True)
    else:
        # x4/wT are bf16 (host-cast). ab is A and B^T concatenated on the
        # free axis so one DMA loads both; declared float32r so the raw fp32
        # bits land ready for full-rate f32r fold matmuls (no cast pass).
        x_in = nc.declare_dram_parameter(
            "x4", [N_TT, P, N_KT, P], dt.bfloat16, isOutput=False
        )
        w_in = nc.declare_dram_parameter(
            "wT", [D_IN, D_OUT], dt.bfloat16, isOutput=False
        )
        ab_in = nc.declare_dram_parameter(
            "ab", [R, D_IN + D_OUT], dt.float32r, isOutput=False
        )
        bias_in = nc.declare_dram_parameter("bias", [D_OUT], dt.float32, isOutput=False)
        out_ext = nc.declare_dram_parameter(
            "out", [T_CORE, D_OUT], dt.float32, isOutput=True
        )

    # Warm-start groups: (tt, oc) psum groups opened during the W load so the
    # PE consumes each W k-tile as it arrives instead of idling behind the
    # weight DMA stream (PE executes in emission order).
    WARM = ((0, 0), (0, 1), (0, 2), (0, 3), (1, 0), (1, 1))

    def prologue(tc, pools, warm=False):
        const, wpool, wstage, xpool, opool, psumF, psumM = pools
        # --- A | B^T in one DMA (f32r bits straight from DRAM) ---
        ab_sb = const.tile([R, D_IN + D_OUT], dt.float32r, tag="ab_sb")
        nc.scalar.dma_start(out=ab_sb[:], in_=ab_in[:])
        a_sb = ab_sb[:, :D_IN]
        bt_sb = ab_sb[:, D_IN:]

        # bias row: one 8KB descriptor into partition 0
        bias_row = const.tile([1, D_OUT], dt.float32, tag="bias_row")
        bias_ap = bias_in[:]
        bias_row_src = bass.AP(
            tensor=bias_ap.tensor,
            offset=bias_ap.offset,
            ap=[[0, 1]] + list(bias_ap.ap),
        )
        nc.scalar.dma_start(out=bias_row[:], in_=bias_row_src)
        ones = const.tile([1, P], dt.float32, tag="ones")
        nc.vector.memset(ones[:], 1.0)
        bias_sb = const.tile([P, D_OUT], dt.float32, tag="bias_sb")

        if warm:
            warm_psums = {
                (tt, oc): psumM.tile(
                    [P, OC_W], dt.float32, tag="psM", name=f"wps_{tt}_{oc}"
                )
                for (tt, oc) in WARM
            }
        else:
            xts, warm_psums = None, None

        # --- W_effT: HWDGE-load W_baseT bf16, fold LoRA + round via DVE,
        # and feed the warm-start groups — all interleaved per k-tile so the
        # PE tracks the W DMA stream. k-tile 0 is DMA'd in OC_W-wide chunks
        # so the first fold matmul starts early; the warm x tiles ride the
        # ACT ring in parallel.
        w_tiles = []
        for kt in range(N_KT):
            ws = wstage.tile([P, D_OUT], dt.bfloat16, tag="ws")
            nc.sync.dma_start(out=ws[:], in_=w_in[kt * P : (kt + 1) * P, :])
            wt = wpool.tile([P, D_OUT], dt.bfloat16, tag=f"w{kt}")
            w_tiles.append(wt)
            for oc in range(N_OC):
                ps = psumF.tile([P, OC_W], dt.float32, tag="psF")
                nc.tensor.matmul(
                    ps[:],
                    a_sb[:, kt * P : (kt + 1) * P],
                    bt_sb[:, oc * OC_W : (oc + 1) * OC_W],
                    start=True,
                    stop=True,
                )
                # W_eff = round_bf16(W_base + SCALE * (BA)^T); SCALE == 1.0
                nc.vector.tensor_add(
                    out=wt[:, oc * OC_W : (oc + 1) * OC_W],
                    in0=ps[:],
                    in1=ws[:, oc * OC_W : (oc + 1) * OC_W],
                )
            if kt == 0:
                if warm:
                    # early x tiles for warm-start groups (ACT ring — off
                    # the W ring, queued behind W k-tile 0 only)
                    xt0 = xpool.tile([P, N_KT, P], dt.bfloat16, tag="xt")
                    nc.scalar.dma_start(out=xt0[:], in_=x_in[0])
                    xt1 = xpool.tile([P, N_KT, P], dt.bfloat16, tag="xt")
                    nc.scalar.dma_start(out=xt1[:], in_=x_in[1])
                    xts = (xt0, xt1)
                # broadcast bias across partitions: ones[1,P].T @ bias_row
                for oc in range(N_OC):
                    bps = psumF.tile([P, OC_W], dt.float32, tag="psF")
                    nc.tensor.matmul(
                        bps[:],
                        ones[:],
                        bias_row[:, oc * OC_W : (oc + 1) * OC_W],
                        start=True,
                        stop=True,
                    )
                    nc.vector.tensor_copy(
                        bias_sb[:, oc * OC_W : (oc + 1) * OC_W], bps[:]
                    )
            if warm:
                for (tt, oc) in WARM:
                    nc.tensor.matmul(
                        warm_psums[(tt, oc)][:],
                        xts[tt][:, kt, :],
                        wt[:, oc * OC_W : (oc + 1) * OC_W],
                        start=(kt == 0),
                        stop=(kt == N_KT - 1),
                    )
        return w_tiles, bias_sb, xts, warm_psums

    def evict(tt, oc, ps, opool, bias_sb, include_outdma):
        ot = opool.tile([P, OC_W], dt.float32, tag="ot")
        nc.vector.tensor_add(
            out=ot[:],
            in0=ps[:],
            in1=bias_sb[:, oc * OC_W : (oc + 1) * OC_W],
        )
        if include_outdma or tt == 0:
            # alternate stores across both HWDGE rings
            eng = nc.scalar if (tt * N_OC + oc) % 2 == 0 else nc.sync
            eng.dma_start(
                out=out_ext[tt * P : (tt + 1) * P, oc * OC_W : (oc + 1) * OC_W],
                in_=ot[:],
            )

    def main_loop(
        tc,
        pools,
        w_tiles,
        bias_sb,
        xts=None,
        warm_psums=None,
        include_xdma=True,
        include_outdma=True,
    ):
        const, wpool, wstage, xpool, opool, psumF, psumM = pools
        warm = warm_psums is not None
        if warm:
            for (tt, oc) in WARM:
                evict(tt, oc, warm_psums[(tt, oc)], opool, bias_sb, include_outdma)
        # --- main GEMM over token tiles ---
        # kt-outer / oc-inner: one stationary load of x[k,t] per k-tile
        # feeds the 4 output-chunk matmuls (4 MMs per LDWEIGHTS).
        xt_fixed = None
        for tt in range(N_TT):
            if warm and tt < len(xts):
                xt = xts[tt]
            elif include_xdma or tt == 0:
                # plain HWDGE: sync ring is idle once W is in
                xt = xpool.tile([P, N_KT, P], dt.bfloat16, tag="xt")
                nc.sync.dma_start(out=xt[:], in_=x_in[tt])
                xt_fixed = xt
            else:
                xt = xt_fixed
            ocs = [oc for oc in range(N_OC) if not (warm and (tt, oc) in WARM)]
            if not ocs:
                continue
            pss = {
                oc: psumM.tile(
                    [P, OC_W], dt.float32, tag="psM", name=f"ps_{tt}_{oc}"
                )
                for oc in ocs
            }
            for kt in range(N_KT):
                for oc in ocs:
                    nc.tensor.matmul(
                        pss[oc][:],
                        xt[:, kt, :],
                        w_tiles[kt][:, oc * OC_W : (oc + 1) * OC_W],
                        start=(kt == 0),
                        stop=(kt == N_KT - 1),
                    )
            for oc in ocs:
                evict(tt, oc, pss[oc], opool, bias_sb, include_outdma)

    with tile.TileContext(nc) as tc:
        with (
            tc.tile_pool(name="const", bufs=1) as const,
            tc.tile_pool(name="wpool", bufs=1) as wpool,
            tc.tile_pool(name="wstage", bufs=6) as wstage,
            tc.tile_pool(name="xpool", bufs=3) as xpool,
            tc.tile_pool(name="opool", bufs=3) as opool,
            tc.tile_pool(name="psumF", bufs=2, space="PSUM") as psumF,
            tc.tile_pool(name="psumM", bufs=6, space="PSUM") as psumM,
        ):
            pools = (const, wpool, wstage, xpool, opool, psumF, psumM)
            if n_iters == 1:
                assert scope == "full"
                w_tiles, bias_sb, xts, wp = prologue(tc, pools, warm=True)
                main_loop(tc, pools, w_tiles, bias_sb, xts, wp)
            elif scope == "full":
                with tc.For_i(0, n_iters, 1):
                    w_tiles, bias_sb, xts, wp = prologue(tc, pools, warm=True)
                    main_loop(tc, pools, w_tiles, bias_sb, xts, wp)
            else:
                w_tiles, bias_sb, _, _ = prologue(tc, pools, warm=False)
                with tc.For_i(0, n_iters, 1):
                    main_loop(
                        tc,
                        pools,
                        w_tiles,
                        bias_sb,
                        include_xdma=(scope != "pe"),
                        include_outdma=(scope != "pe"),
                    )
            if internal_io:
                tk = const.tile([P, 4], dt.float32, tag="tick")
                nc.sync.dma_start(out=tk[:], in_=dummy_in[:])
                nc.sync.dma_start(out=dummy_out[:], in_=tk[:])
    nc.compile()
    return nc


class _SpmdRunner:
    """Mirrors concourse.bass2jax.run_bass_via_pjrt but keeps the jitted
    executable alive so repeated calls don't recompile."""

    def __init__(self, nc, n_cores: int):
        import jax
        from jax.sharding import Mesh, PartitionSpec
        from jax.experimental.shard_map import shard_map
        import concourse.mybir as mybir
        from concourse.bass2jax import (
            _bass_exec_p,
            install_neuronx_cc_hook,
            partition_id_tensor,
        )

        install_neuronx_cc_hook()
        self.nc = nc
        self.n_cores = n_cores
        partition_name = (
            nc.partition_id_tensor.name if nc.partition_id_tensor else None
        )
        in_names, out_names, out_avals, zero_shapes = [], [], [], []
        for alloc in nc.m.functions[0].allocations:
            if not isinstance(alloc, mybir.MemoryLocationSet):
                continue
            name = alloc.memorylocations[0].name
            if alloc.kind == "ExternalInput":
                if name != partition_name:
                    in_names.append(name)
            elif alloc.kind == "ExternalOutput":
                shape = tuple(alloc.tensor_shape)
                dtype = mybir.dt.np(alloc.dtype)
                out_names.append(name)
                out_avals.append(jax.core.ShapedArray(shape, dtype))
                zero_shapes.append((shape, dtype))
        self.in_param_names = list(in_names)
        self.out_names = out_names
        self.out_avals = tuple(out_avals)
        self.zero_shapes = zero_shapes
        n_params = len(in_names)
        all_in_names = in_names + out_names
        if partition_name is not None:
            all_in_names.append(partition_name)
        n_outs = len(out_names)
        donate = tuple(range(n_params, n_params + n_outs))

        def _body(*args):
            operands = list(args)
            if partition_name is not None:
                operands.append(partition_id_tensor())
            outs = _bass_exec_p.bind(
                *operands,
                out_avals=self.out_avals,
                in_names=tuple(all_in_names),
                out_names=tuple(out_names),
                lowering_input_output_aliases=(),
                sim_require_finite=True,
                sim_require_nnan=True,
                nc=nc,
            )
            return tuple(outs)

        devices = jax.devices()[:n_cores]
        assert len(devices) == n_cores, (
            f"need {n_cores} neuron cores, found {len(jax.devices())}"
        )
        mesh = Mesh(np.asarray(devices), ("core",))
        in_specs = (PartitionSpec("core"),) * (n_params + n_outs)
        out_specs = (PartitionSpec("core"),) * n_outs
        self.sharded = jax.jit(
            shard_map(
                _body,
                mesh=mesh,
                in_specs=in_specs,
                out_specs=out_specs,
                check_rep=False,
            ),
            donate_argnums=donate,
            keep_unused=True,
        )

    def concat_inputs(self, in_maps):
        return [
            np.concatenate(
                [np.asarray(in_maps[c][n]) for c in range(self.n_cores)], axis=0
            )
            for n in self.in_param_names
        ]

    def _zeros(self):
        return [
            np.zeros((self.n_cores * s[0], *s[1:]), d)
            for (s, d) in self.zero_shapes
        ]

    def run_concat(self, concat_in):
        return self.sharded(*concat_in, *self._zeros())

    def run(self, in_maps):
        out_arrs = self.run_concat(self.concat_inputs(in_maps))
        res = []
        for c in range(self.n_cores):
            m = {}
            for i, name in enumerate(self.out_names):
                s = self.out_avals[i].shape
                m[name] = np.asarray(out_arrs[i]).reshape(self.n_cores, *s)[c]
            res.append(m)
        return res


def get_runner(n_iters: int = 1):
    key = ("runner", n_iters)
    if key not in _CACHE:
        nc = _build_program(n_iters=n_iters)
        _CACHE[key] = _SpmdRunner(nc, N_CORES)
    return _CACHE[key]


def make_in_maps(x, W_base, b_base, A, B):
    bf16 = ml_dtypes.bfloat16
    x2d = np.ascontiguousarray(x, dtype=np.float32).reshape(B_ * S, D_IN)
    wT = np.ascontiguousarray(W_base.T.astype(bf16))
    ab = np.ascontiguousarray(
        np.concatenate([A, B.T], axis=1), dtype=np.float32
    )
    bias = np.ascontiguousarray(b_base, dtype=np.float32)
    in_maps = []
    for c in range(N_CORES):
        xc = x2d[c * T_CORE : (c + 1) * T_CORE]  # [2048 t, 2048 k]
        # x4[tt, p(k), kt, t] = xc[tt*128 + t, kt*128 + p]  (SBUF layout)
        x4 = np.ascontiguousarray(
            xc.reshape(N_TT, P, N_KT, P).transpose(0, 3, 2, 1).astype(bf16)
        )
        in_maps.append({"x4": x4, "wT": wT, "ab": ab, "bias": bias})
    return in_maps


def kernel(**inputs):
    x = inputs["x"]
    W_base = inputs["W_base"]
    b_base = inputs["b_base"]
    A = inputs["A"]
    B = inputs["B"]
    runner = get_runner()
    in_maps = make_in_maps(x, W_base, b_base, A, B)
    res = runner.run(in_maps)
    y2d = np.concatenate([res[c]["out"] for c in range(N_CORES)], axis=0)
    return np.ascontiguousarray(y2d.reshape(B_, S, D_OUT), dtype=np.float32)



# revision 13
# speedup vs baseline: 1.0633x; 1.0564x over previous
"""Trainium2 Bass kernel for CrossLoRALinear:
    y = x @ W_base^T + b_base + ((x @ A^T) @ B^T) * SCALE

Strategy (8 NeuronCores, SPMD):
  - Data-parallel shard of the 4*4096=16384 tokens: 2048 tokens/core.
  - Replicate W_base/A/B/b_base.
  - Host casts x and W_baseT to bf16 (output error ~1e-3 vs the 2e-2 gate):
    halves HBM traffic and SBUF footprint; PE rate is the same as f32r.
  - On device, fold the rank-16 LoRA update into the weights once:
        W_effT[k,o] = round_bf16(W_baseT[k,o] + SCALE * (A^T @ B^T)[k,o])
    (64 K=16 matmuls + DVE adds), keeping W_effT resident in SBUF.
  - bias is broadcast across partitions with a K=1 PE matmul against a
    ones row (avoids a 128-descriptor broadcast DMA on the critical
    prologue stream).
  - Main GEMM per core: out[t,o] = sum_k xT[k,t]^T @ W_effT[k,o], fp32
    PSUM accumulation, bias fused into the PSUM->SBUF eviction (DVE).
  - Warm-start groups overlap the main GEMM with the W-load stream.
  - For_i timing loops arm the PE branch prefetcher (hint_engines) so the
    back-edge I$-hits instead of stalling on an IRAM refetch.
  - Host does layout only: shard/transpose/cast x, transpose W/B, concat.
"""
import sys

if "/opt/trn_rl_repo" not in sys.path:
    sys.path.insert(0, "/opt/trn_rl_repo")

import numpy as np
import ml_dtypes

N_CORES = 8
B_, S, D_IN, D_OUT, R = 4, 4096, 2048, 2048, 16
SCALE = 1.0
T_CORE = (B_ * S) // N_CORES  # 2048 tokens per core
P = 128
N_TT = T_CORE // P   # 16 token tiles per core
N_KT = D_IN // P     # 16 contraction tiles
OC_W = 512           # psum bank width (fp32)
N_OC = D_OUT // OC_W # 4 output chunks

_CACHE = {}


def _build_program(
    n_iters: int = 1,
    internal_io: bool = False,
    scope: str = "full",
):
    import concourse.bacc as bacc
    import concourse.mybir as mybir
    import concourse.bass as bass
    from concourse import tile

    dt = mybir.dt
    nc = bacc.Bacc(None, target_bir_lowering=False, debug=False)

    if internal_io:
        # Timing-only variant: big tensors live in device DRAM (contents
        # irrelevant for dense GEMM timing); tiny external I/O keeps the
        # graph alive and dispatch overhead minimal.
        x_in = nc.dram_tensor("x4_d", [N_TT, P, N_KT, P], dt.bfloat16)[:]
        w_in = nc.dram_tensor("wT_d", [D_IN, D_OUT], dt.bfloat16)[:]
        ab_in = nc.dram_tensor("ab_d", [R, D_IN + D_OUT], dt.float32r)[:]
        bias_in = nc.dram_tensor("bias_d", [D_OUT], dt.float32)[:]
        out_ext = nc.dram_tensor("out_d", [T_CORE, D_OUT], dt.float32)[:]
        dummy_in = nc.declare_dram_parameter("tick", [P, 4], dt.float32, isOutput=False)
        dummy_out = nc.declare_dram_parameter("tock", [P, 4], dt.float32, isOutput=True)
    else:
        # x4/wT are bf16 (host-cast). ab is A and B^T concatenated on the
        # free axis so one DMA loads both; declared float32r so the raw fp32
        # bits land ready for full-rate f32r fold matmuls (no cast pass).
        x_in = nc.declare_dram_parameter(
            "x4", [N_TT, P, N_KT, P], dt.bfloat16, isOutput=False
        )
        w_in = nc.declare_dram_parameter(
            "wT", [D_IN, D_OUT], dt.bfloat16, isOutput=False
        )
        ab_in = nc.declare_dram_parameter(
            "ab", [R, D_IN + D_OUT], dt.float32r, isOutput=False
        )
        bias_in = nc.declare_dram_parameter("bias", [D_OUT], dt.float32, isOutput=False)
        out_ext = nc.declare_dram_parameter(
            "out", [T_CORE, D_OUT], dt.float32, isOutput=True
        )

    # Warm-start groups: (tt, oc) psum groups opened during the W load so the
    # PE consumes each W k-tile as it arrives instead of idling behind the
    # weight DMA stream (PE executes in emission order).
    WARM = ((0, 0), (0, 1), (0, 2), (0, 3), (1, 0), (1, 1))

    def prologue(tc, pools, warm=False):
        const, wpool, wstage, xpool, opool, psumF, psumM = pools
        # --- A | B^T in one DMA (f32r bits straight from DRAM) ---
        ab_sb = const.tile([R, D_IN + D_OUT], dt.float32r, tag="ab_sb")
        nc.scalar.dma_start(out=ab_sb[:], in_=ab_in[:])
        a_sb = ab_sb[:, :D_IN]
        bt_sb = ab_sb[:, D_IN:]

        # bias row: one 8KB descriptor into partition 0
        bias_row = const.tile([1, D_OUT], dt.float32, tag="bias_row")
        bias_ap = bias_in[:]
        bias_row_src = bass.AP(
            tensor=bias_ap.tensor,
            offset=bias_ap.offset,
            ap=[[0, 1]] + list(bias_ap.ap),
        )
        nc.scalar.dma_start(out=bias_row[:], in_=bias_row_src)
        ones = const.tile([1, P], dt.float32, tag="ones")
        nc.vector.memset(ones[:], 1.0)
        bias_sb = const.tile([P, D_OUT], dt.float32, tag="bias_sb")

        if warm:
            warm_psums = {
                (tt, oc): psumM.tile(
                    [P, OC_W], dt.float32, tag="psM", name=f"wps_{tt}_{oc}"
                )
                for (tt, oc) in WARM
            }
        else:
            xts, warm_psums = None, None

        # --- W_effT: HWDGE-load W_baseT bf16, fold LoRA + round via DVE,
        # and feed the warm-start groups — all interleaved per k-tile so the
        # PE tracks the W DMA stream. k-tile 0 is DMA'd in OC_W-wide chunks
        # so the first fold matmul starts early; the warm x tiles ride the
        # ACT ring in parallel.
        w_tiles = []
        for kt in range(N_KT):
            ws = wstage.tile([P, D_OUT], dt.bfloat16, tag="ws")
            nc.sync.dma_start(out=ws[:], in_=w_in[kt * P : (kt + 1) * P, :])
            wt = wpool.tile([P, D_OUT], dt.bfloat16, tag=f"w{kt}")
            w_tiles.append(wt)
            for oc in range(N_OC):
                ps = psumF.tile([P, OC_W], dt.float32, tag="psF")
                nc.tensor.matmul(
                    ps[:],
                    a_sb[:, kt * P : (kt + 1) * P],
                    bt_sb[:, oc * OC_W : (oc + 1) * OC_W],
                    start=True,
                    stop=True,
                )
                # W_eff = round_bf16(W_base + SCALE * (BA)^T); SCALE == 1.0
                nc.vector.tensor_add(
                    out=wt[:, oc * OC_W : (oc + 1) * OC_W],
                    in0=ps[:],
                    in1=ws[:, oc * OC_W : (oc + 1) * OC_W],
                )
            if kt == 0:
                if warm:
                    # early x tiles for warm-start groups (ACT ring — off
                    # the W ring, queued behind W k-tile 0 only)
                    xt0 = xpool.tile([P, N_KT, P], dt.bfloat16, tag="xt")
                    nc.scalar.dma_start(out=xt0[:], in_=x_in[0])
                    xt1 = xpool.tile([P, N_KT, P], dt.bfloat16, tag="xt")
                    nc.scalar.dma_start(out=xt1[:], in_=x_in[1])
                    xts = (xt0, xt1)
                # broadcast bias across partitions: ones[1,P].T @ bias_row
                for oc in range(N_OC):
                    bps = psumF.tile([P, OC_W], dt.float32, tag="psF")
                    nc.tensor.matmul(
                        bps[:],
                        ones[:],
                        bias_row[:, oc * OC_W : (oc + 1) * OC_W],
                        start=True,
                        stop=True,
                    )
                    nc.vector.tensor_copy(
                        bias_sb[:, oc * OC_W : (oc + 1) * OC_W], bps[:]
                    )
            if warm:
                for (tt, oc) in WARM:
                    nc.tensor.matmul(
                        warm_psums[(tt, oc)][:],
                        xts[tt][:, kt, :],
                        wt[:, oc * OC_W : (oc + 1) * OC_W],
                        start=(kt == 0),
                        stop=(kt == N_KT - 1),
                    )
        return w_tiles, bias_sb, xts, warm_psums

    def evict(tt, oc, ps, opool, bias_sb, include_outdma):
        ot = opool.tile([P, OC_W], dt.float32, tag="ot")
        nc.vector.tensor_add(
            out=ot[:],
            in0=ps[:],
            in1=bias_sb[:, oc * OC_W : (oc + 1) * OC_W],
        )
        if include_outdma or tt == 0:
            # ACT-ring HWDGE: keeps stores off the SP ring
            nc.scalar.dma_start(
                out=out_ext[tt * P : (tt + 1) * P, oc * OC_W : (oc + 1) * OC_W],
                in_=ot[:],
            )

    def main_loop(
        tc,
        pools,
        w_tiles,
        bias_sb,
        xts=None,
        warm_psums=None,
        include_xdma=True,
        include_outdma=True,
    ):
        const, wpool, wstage, xpool, opool, psumF, psumM = pools
        warm = warm_psums is not None
        if warm:
            for (tt, oc) in WARM:
                evict(tt, oc, warm_psums[(tt, oc)], opool, bias_sb, include_outdma)
        # --- main GEMM over token tiles ---
        xt_fixed = None
        for tt in range(N_TT):
            if warm and tt < len(xts):
                xt = xts[tt]
            elif include_xdma or tt == 0:
                # plain HWDGE: sync ring is idle once W is in
                xt = xpool.tile([P, N_KT, P], dt.bfloat16, tag="xt")
                nc.sync.dma_start(out=xt[:], in_=x_in[tt])
                xt_fixed = xt
            else:
                xt = xt_fixed
            for oc in range(N_OC):
                if warm and (tt, oc) in WARM:
                    continue
                ps = psumM.tile([P, OC_W], dt.float32, tag="psM")
                for kt in range(N_KT):
                    nc.tensor.matmul(
                        ps[:],
                        xt[:, kt, :],
                        w_tiles[kt][:, oc * OC_W : (oc + 1) * OC_W],
                        start=(kt == 0),
                        stop=(kt == N_KT - 1),
                    )
                evict(tt, oc, ps, opool, bias_sb, include_outdma)

    with tile.TileContext(nc) as tc:
        with (
            tc.tile_pool(name="const", bufs=1) as const,
            tc.tile_pool(name="wpool", bufs=1) as wpool,
            tc.tile_pool(name="wstage", bufs=6) as wstage,
            tc.tile_pool(name="xpool", bufs=3) as xpool,
            tc.tile_pool(name="opool", bufs=3) as opool,
            tc.tile_pool(name="psumF", bufs=2, space="PSUM") as psumF,
            tc.tile_pool(name="psumM", bufs=6, space="PSUM") as psumM,
        ):
            pools = (const, wpool, wstage, xpool, opool, psumF, psumM)
            if n_iters == 1:
                assert scope == "full"
                w_tiles, bias_sb, xts, wp = prologue(tc, pools, warm=True)
                main_loop(tc, pools, w_tiles, bias_sb, xts, wp)
            elif scope == "full":
                with tc.For_i(0, n_iters, 1, hint_engines=(mybir.EngineType.PE,)):
                    w_tiles, bias_sb, xts, wp = prologue(tc, pools, warm=True)
                    main_loop(tc, pools, w_tiles, bias_sb, xts, wp)
            else:
                w_tiles, bias_sb, _, _ = prologue(tc, pools, warm=False)
                with tc.For_i(0, n_iters, 1, hint_engines=(mybir.EngineType.PE,)):
                    main_loop(
                        tc,
                        pools,
                        w_tiles,
                        bias_sb,
                        include_xdma=(scope != "pe"),
                        include_outdma=(scope != "pe"),
                    )
            if internal_io:
                tk = const.tile([P, 4], dt.float32, tag="tick")
                nc.sync.dma_start(out=tk[:], in_=dummy_in[:])
                nc.sync.dma_start(out=dummy_out[:], in_=tk[:])
    nc.compile()
    return nc


class _SpmdRunner:
    """Mirrors concourse.bass2jax.run_bass_via_pjrt but keeps the jitted
    executable alive so repeated calls don't recompile."""

    def __init__(self, nc, n_cores: int):
        import jax
        from jax.sharding import Mesh, PartitionSpec
        from jax.experimental.shard_map import shard_map
        import concourse.mybir as mybir
        from concourse.bass2jax import (
            _bass_exec_p,
            install_neuronx_cc_hook,
            partition_id_tensor,
        )

        install_neuronx_cc_hook()
        self.nc = nc
        self.n_cores = n_cores
        partition_name = (
            nc.partition_id_tensor.name if nc.partition_id_tensor else None
        )
        in_names, out_names, out_avals, zero_shapes = [], [], [], []
        for alloc in nc.m.functions[0].allocations:
            if not isinstance(alloc, mybir.MemoryLocationSet):
                continue
            name = alloc.memorylocations[0].name
            if alloc.kind == "ExternalInput":
                if name != partition_name:
                    in_names.append(name)
            elif alloc.kind == "ExternalOutput":
                shape = tuple(alloc.tensor_shape)
                dtype = mybir.dt.np(alloc.dtype)
                out_names.append(name)
                out_avals.append(jax.core.ShapedArray(shape, dtype))
                zero_shapes.append((shape, dtype))
        self.in_param_names = list(in_names)
        self.out_names = out_names
        self.out_avals = tuple(out_avals)
        self.zero_shapes = zero_shapes
        n_params = len(in_names)
        all_in_names = in_names + out_names
        if partition_name is not None:
            all_in_names.append(partition_name)
        n_outs = len(out_names)
        donate = tuple(range(n_params, n_params + n_outs))

        def _body(*args):
            operands = list(args)
            if partition_name is not None:
                operands.append(partition_id_tensor())
            outs = _bass_exec_p.bind(
                *operands,
                out_avals=self.out_avals,
                in_names=tuple(all_in_names),
                out_names=tuple(out_names),
                lowering_input_output_aliases=(),
                sim_require_finite=True,
                sim_require_nnan=True,
                nc=nc,
            )
            return tuple(outs)

        devices = jax.devices()[:n_cores]
        assert len(devices) == n_cores, (
            f"need {n_cores} neuron cores, found {len(jax.devices())}"
        )
        mesh = Mesh(np.asarray(devices), ("core",))
        in_specs = (PartitionSpec("core"),) * (n_params + n_outs)
        out_specs = (PartitionSpec("core"),) * n_outs
        self.sharded = jax.jit(
            shard_map(
                _body,
                mesh=mesh,
                in_specs=in_specs,
                out_specs=out_specs,
                check_rep=False,
            ),
            donate_argnums=donate,
            keep_unused=True,
        )

    def concat_inputs(self, in_maps):
        return [
            np.concatenate(
                [np.asarray(in_maps[c][n]) for c in range(self.n_cores)], axis=0
            )
            for n in self.in_param_names
        ]

    def _zeros(self):
        return [
            np.zeros((self.n_cores * s[0], *s[1:]), d)
            for (s, d) in self.zero_shapes
        ]

    def run_concat(self, concat_in):
        return self.sharded(*concat_in, *self._zeros())

    def run(self, in_maps):
        out_arrs = self.run_concat(self.concat_inputs(in_maps))
        res = []
        for c in range(self.n_cores):
            m = {}
            for i, name in enumerate(self.out_names):
                s = self.out_avals[i].shape
                m[name] = np.asarray(out_arrs[i]).reshape(self.n_cores, *s)[c]
            res.append(m)
        return res


def get_runner(n_iters: int = 1):
    key = ("runner", n_iters)
    if key not in _CACHE:
        nc = _build_program(n_iters=n_iters)
        _CACHE[key] = _SpmdRunner(nc, N_CORES)
    return _CACHE[key]


def make_in_maps(x, W_base, b_base, A, B):
    bf16 = ml_dtypes.bfloat16
    x2d = np.ascontiguousarray(x, dtype=np.float32).reshape(B_ * S, D_IN)
    wT = np.ascontiguousarray(W_base.T.astype(bf16))
    ab = np.ascontiguousarray(
        np.concatenate([A, B.T], axis=1), dtype=np.float32
    )
    bias = np.ascontiguousarray(b_base, dtype=np.float32)
    in_maps = []
    for c in range(N_CORES):
        xc = x2d[c * T_CORE : (c + 1) * T_CORE]  # [2048 t, 2048 k]
        # x4[tt, p(k), kt, t] = xc[tt*128 + t, kt*128 + p]  (SBUF layout)
        x4 = np.ascontiguousarray(
            xc.reshape(N_TT, P, N_KT, P).transpose(0, 3, 2, 1).astype(bf16)
        )
        in_maps.append({"x4": x4, "wT": wT, "ab": ab, "bias": bias})
    return in_maps


def kernel(**inputs):
    x = inputs["x"]
    W_base = inputs["W_base"]
    b_base = inputs["b_base"]
    A = inputs["A"]
    B = inputs["B"]
    runner = get_runner()
    in_maps = make_in_maps(x, W_base, b_base, A, B)
    res = runner.run(in_maps)
    y2d = np.concatenate([res[c]["out"] for c in range(N_CORES)], axis=0)
    return np.ascontiguousarray(y2d.reshape(B_, S, D_OUT), dtype=np.float32)



# revision 14
# speedup vs baseline: 1.4342x; 1.3489x over previous
"""Trainium2 Bass kernel for CrossLoRALinear:
    y = x @ W_base^T + b_base + ((x @ A^T) @ B^T) * SCALE

Strategy (8 NeuronCores, SPMD):
  - Data-parallel shard of the 4*4096=16384 tokens: 2048 tokens/core.
  - Replicate W_base/A/B/b_base.
  - Host casts x and W_baseT to bf16 (output error ~1e-3 vs the 2e-2 gate):
    halves HBM traffic and SBUF footprint; PE rate is the same as f32r.
  - On device, fold the rank-16 LoRA update into the weights once:
        W_effT[k,o] = round_bf16(W_baseT[k,o] + SCALE * (A^T @ B^T)[k,o])
    (64 K=16 matmuls + DVE adds), keeping W_effT resident in SBUF.
  - bias is broadcast across partitions with a K=1 PE matmul against a
    ones row (avoids a 128-descriptor broadcast DMA on the critical
    prologue stream).
  - Main GEMM per core: out[t,o] = sum_k xT[k,t]^T @ W_effT[k,o], fp32
    PSUM accumulation, bias fused into the PSUM->SBUF eviction (DVE).
  - PSUM pair-tiling: each psum tile spans TWO banks (one oc-pair,
    [128,1024] fp32); the two 512-wide halves accumulate as independent
    matmul groups, then ONE wide DVE add evicts both and ONE 512 KB DMA
    stores them (halves eviction instructions, store count, and sem hops
    -- measured -40..-50 us vs single-bank groups).
  - Warm-start groups overlap the main GEMM with the W-load stream.
  - Host does layout only: shard/transpose/cast x, transpose W/B, concat.
"""
import sys

if "/opt/trn_rl_repo" not in sys.path:
    sys.path.insert(0, "/opt/trn_rl_repo")

import numpy as np
import ml_dtypes

N_CORES = 8
B_, S, D_IN, D_OUT, R = 4, 4096, 2048, 2048, 16
SCALE = 1.0
T_CORE = (B_ * S) // N_CORES  # 2048 tokens per core
P = 128
N_TT = T_CORE // P   # 16 token tiles per core
N_KT = D_IN // P     # 16 contraction tiles
OC_W = 512           # psum bank width (fp32)
N_OC = D_OUT // OC_W # 4 output chunks

_CACHE = {}


def _build_program(
    n_iters: int = 1,
    internal_io: bool = False,
    scope: str = "full",
):
    import concourse.bacc as bacc
    import concourse.mybir as mybir
    import concourse.bass as bass
    from concourse import tile

    dt = mybir.dt
    nc = bacc.Bacc(None, target_bir_lowering=False, debug=False)

    if internal_io:
        # Timing-only variant: big tensors live in device DRAM (contents
        # irrelevant for dense GEMM timing); tiny external I/O keeps the
        # graph alive and dispatch overhead minimal.
        x_in = nc.dram_tensor("x4_d", [N_TT, P, N_KT, P], dt.bfloat16)[:]
        w_in = nc.dram_tensor("wT_d", [D_IN, D_OUT], dt.bfloat16)[:]
        ab_in = nc.dram_tensor("ab_d", [R, D_IN + D_OUT], dt.float32r)[:]
        bias_in = nc.dram_tensor("bias_d", [D_OUT], dt.float32)[:]
        out_ext = nc.dram_tensor("out_d", [T_CORE, D_OUT], dt.float32)[:]
        dummy_in = nc.declare_dram_parameter("tick", [P, 4], dt.float32, isOutput=False)
        dummy_out = nc.declare_dram_parameter("tock", [P, 4], dt.float32, isOutput=True)
    else:
        # x4/wT are bf16 (host-cast). ab is A and B^T concatenated on the
        # free axis so one DMA loads both; declared float32r so the raw fp32
        # bits land ready for full-rate f32r fold matmuls (no cast pass).
        x_in = nc.declare_dram_parameter(
            "x4", [N_TT, P, N_KT, P], dt.bfloat16, isOutput=False
        )
        w_in = nc.declare_dram_parameter(
            "wT", [D_IN, D_OUT], dt.bfloat16, isOutput=False
        )
        ab_in = nc.declare_dram_parameter(
            "ab", [R, D_IN + D_OUT], dt.float32r, isOutput=False
        )
        bias_in = nc.declare_dram_parameter("bias", [D_OUT], dt.float32, isOutput=False)
        out_ext = nc.declare_dram_parameter(
            "out", [T_CORE, D_OUT], dt.float32, isOutput=True
        )

    # Warm-start groups: (tt, oc) psum groups opened during the W load so the
    # PE consumes each W k-tile as it arrives instead of idling behind the
    # weight DMA stream (PE executes in emission order).
    WARM = ((0, 0), (0, 1), (1, 0))  # (tt, oc-pair): pair p covers oc 2p, 2p+1

    def prologue(tc, pools, warm=False):
        const, wpool, wstage, xpool, opool, psumF, psumM = pools
        # --- A | B^T in one DMA (f32r bits straight from DRAM) ---
        ab_sb = const.tile([R, D_IN + D_OUT], dt.float32r, tag="ab_sb")
        nc.scalar.dma_start(out=ab_sb[:], in_=ab_in[:])
        a_sb = ab_sb[:, :D_IN]
        bt_sb = ab_sb[:, D_IN:]

        # bias row: one 8KB descriptor into partition 0
        bias_row = const.tile([1, D_OUT], dt.float32, tag="bias_row")
        bias_ap = bias_in[:]
        bias_row_src = bass.AP(
            tensor=bias_ap.tensor,
            offset=bias_ap.offset,
            ap=[[0, 1]] + list(bias_ap.ap),
        )
        nc.scalar.dma_start(out=bias_row[:], in_=bias_row_src)
        ones = const.tile([1, P], dt.float32, tag="ones")
        nc.vector.memset(ones[:], 1.0)
        bias_sb = const.tile([P, D_OUT], dt.float32, tag="bias_sb")

        if warm:
            warm_psums = {
                (tt, op): psumM.tile(
                    [P, 2 * OC_W], dt.float32, tag="psM", name=f"wps_{tt}_{op}"
                )
                for (tt, op) in WARM
            }
        else:
            xts, warm_psums = None, None

        # --- W_effT: HWDGE-load W_baseT bf16, fold LoRA + round via DVE,
        # and feed the warm-start groups — all interleaved per k-tile so the
        # PE tracks the W DMA stream. k-tile 0 is DMA'd in OC_W-wide chunks
        # so the first fold matmul starts early; the warm x tiles ride the
        # ACT ring in parallel.
        w_tiles = []
        for kt in range(N_KT):
            ws = wstage.tile([P, D_OUT], dt.bfloat16, tag="ws")
            nc.sync.dma_start(out=ws[:], in_=w_in[kt * P : (kt + 1) * P, :])
            wt = wpool.tile([P, D_OUT], dt.bfloat16, tag=f"w{kt}")
            w_tiles.append(wt)
            for oc in range(N_OC):
                ps = psumF.tile([P, OC_W], dt.float32, tag="psF")
                nc.tensor.matmul(
                    ps[:],
                    a_sb[:, kt * P : (kt + 1) * P],
                    bt_sb[:, oc * OC_W : (oc + 1) * OC_W],
                    start=True,
                    stop=True,
                )
                # W_eff = round_bf16(W_base + SCALE * (BA)^T); SCALE == 1.0
                nc.vector.tensor_add(
                    out=wt[:, oc * OC_W : (oc + 1) * OC_W],
                    in0=ps[:],
                    in1=ws[:, oc * OC_W : (oc + 1) * OC_W],
                )
            if kt == 0:
                if warm:
                    # early x tiles for warm-start groups (ACT ring — off
                    # the W ring, queued behind W k-tile 0 only)
                    xt0 = xpool.tile([P, N_KT, P], dt.bfloat16, tag="xt")
                    nc.scalar.dma_start(out=xt0[:], in_=x_in[0])
                    xt1 = xpool.tile([P, N_KT, P], dt.bfloat16, tag="xt")
                    nc.scalar.dma_start(out=xt1[:], in_=x_in[1])
                    xts = (xt0, xt1)
                # broadcast bias across partitions: ones[1,P].T @ bias_row
                for oc in range(N_OC):
                    bps = psumF.tile([P, OC_W], dt.float32, tag="psF")
                    nc.tensor.matmul(
                        bps[:],
                        ones[:],
                        bias_row[:, oc * OC_W : (oc + 1) * OC_W],
                        start=True,
                        stop=True,
                    )
                    nc.vector.tensor_copy(
                        bias_sb[:, oc * OC_W : (oc + 1) * OC_W], bps[:]
                    )
            if warm:
                for (tt, op) in WARM:
                    for h in range(2):
                        oc = 2 * op + h
                        nc.tensor.matmul(
                            warm_psums[(tt, op)][:, h * OC_W : (h + 1) * OC_W],
                            xts[tt][:, kt, :],
                            wt[:, oc * OC_W : (oc + 1) * OC_W],
                            start=(kt == 0),
                            stop=(kt == N_KT - 1),
                        )
        return w_tiles, bias_sb, xts, warm_psums

    def evict(tt, op, ps, opool, bias_sb, include_outdma):
        # one 2-bank-wide DVE add + one 512KB store per oc-pair
        ot = opool.tile([P, 2 * OC_W], dt.float32, tag="ot")
        nc.vector.tensor_add(
            out=ot[:],
            in0=ps[:],
            in1=bias_sb[:, 2 * op * OC_W : 2 * (op + 1) * OC_W],
        )
        if include_outdma or tt == 0:
            # ACT-ring HWDGE: keeps stores off the SP ring
            nc.scalar.dma_start(
                out=out_ext[
                    tt * P : (tt + 1) * P, 2 * op * OC_W : 2 * (op + 1) * OC_W
                ],
                in_=ot[:],
            )

    def main_loop(
        tc,
        pools,
        w_tiles,
        bias_sb,
        xts=None,
        warm_psums=None,
        include_xdma=True,
        include_outdma=True,
    ):
        const, wpool, wstage, xpool, opool, psumF, psumM = pools
        warm = warm_psums is not None
        if warm:
            for (tt, op) in WARM:
                evict(tt, op, warm_psums[(tt, op)], opool, bias_sb, include_outdma)
        # --- main GEMM over token tiles ---
        xt_fixed = None
        for tt in range(N_TT):
            if warm and tt < len(xts):
                xt = xts[tt]
            elif include_xdma or tt == 0:
                # plain HWDGE: sync ring is idle once W is in
                xt = xpool.tile([P, N_KT, P], dt.bfloat16, tag="xt")
                nc.sync.dma_start(out=xt[:], in_=x_in[tt])
                xt_fixed = xt
            else:
                xt = xt_fixed
            for op in range(N_OC // 2):
                if warm and (tt, op) in WARM:
                    continue
                ps = psumM.tile([P, 2 * OC_W], dt.float32, tag="psM")
                for kt in range(N_KT):
                    for h in range(2):
                        oc = 2 * op + h
                        nc.tensor.matmul(
                            ps[:, h * OC_W : (h + 1) * OC_W],
                            xt[:, kt, :],
                            w_tiles[kt][:, oc * OC_W : (oc + 1) * OC_W],
                            start=(kt == 0),
                            stop=(kt == N_KT - 1),
                        )
                evict(tt, op, ps, opool, bias_sb, include_outdma)

    with tile.TileContext(nc) as tc:
        with (
            tc.tile_pool(name="const", bufs=1) as const,
            tc.tile_pool(name="wpool", bufs=1) as wpool,
            tc.tile_pool(name="wstage", bufs=6) as wstage,
            tc.tile_pool(name="xpool", bufs=3) as xpool,
            tc.tile_pool(name="opool", bufs=3) as opool,
            tc.tile_pool(name="psumF", bufs=2, space="PSUM") as psumF,
            tc.tile_pool(name="psumM", bufs=3, space="PSUM") as psumM,
        ):
            pools = (const, wpool, wstage, xpool, opool, psumF, psumM)
            if n_iters == 1:
                assert scope == "full"
                w_tiles, bias_sb, xts, wp = prologue(tc, pools, warm=True)
                main_loop(tc, pools, w_tiles, bias_sb, xts, wp)
            elif scope == "full":
                with tc.For_i(0, n_iters, 1, hint_engines=(mybir.EngineType.PE,)):
                    w_tiles, bias_sb, xts, wp = prologue(tc, pools, warm=True)
                    main_loop(tc, pools, w_tiles, bias_sb, xts, wp)
            else:
                w_tiles, bias_sb, _, _ = prologue(tc, pools, warm=False)
                with tc.For_i(0, n_iters, 1, hint_engines=(mybir.EngineType.PE,)):
                    main_loop(
                        tc,
                        pools,
                        w_tiles,
                        bias_sb,
                        include_xdma=(scope != "pe"),
                        include_outdma=(scope != "pe"),
                    )
            if internal_io:
                tk = const.tile([P, 4], dt.float32, tag="tick")
                nc.sync.dma_start(out=tk[:], in_=dummy_in[:])
                nc.sync.dma_start(out=dummy_out[:], in_=tk[:])
    nc.compile()
    return nc


class _SpmdRunner:
    """Mirrors concourse.bass2jax.run_bass_via_pjrt but keeps the jitted
    executable alive so repeated calls don't recompile."""

    def __init__(self, nc, n_cores: int):
        import jax
        from jax.sharding import Mesh, PartitionSpec
        from jax.experimental.shard_map import shard_map
        import concourse.mybir as mybir
        from concourse.bass2jax import (
            _bass_exec_p,
            install_neuronx_cc_hook,
            partition_id_tensor,
        )

        install_neuronx_cc_hook()
        self.nc = nc
        self.n_cores = n_cores
        partition_name = (
            nc.partition_id_tensor.name if nc.partition_id_tensor else None
        )
        in_names, out_names, out_avals, zero_shapes = [], [], [], []
        for alloc in nc.m.functions[0].allocations:
            if not isinstance(alloc, mybir.MemoryLocationSet):
                continue
            name = alloc.memorylocations[0].name
            if alloc.kind == "ExternalInput":
                if name != partition_name:
                    in_names.append(name)
            elif alloc.kind == "ExternalOutput":
                shape = tuple(alloc.tensor_shape)
                dtype = mybir.dt.np(alloc.dtype)
                out_names.append(name)
                out_avals.append(jax.core.ShapedArray(shape, dtype))
                zero_shapes.append((shape, dtype))
        self.in_param_names = list(in_names)
        self.out_names = out_names
        self.out_avals = tuple(out_avals)
        self.zero_shapes = zero_shapes
        n_params = len(in_names)
        all_in_names = in_names + out_names
        if partition_name is not None:
            all_in_names.append(partition_name)
        n_outs = len(out_names)
        donate = tuple(range(n_params, n_params + n_outs))

        def _body(*args):
            operands = list(args)
            if partition_name is not None:
                operands.append(partition_id_tensor())
            outs = _bass_exec_p.bind(
                *operands,
                out_avals=self.out_avals,
                in_names=tuple(all_in_names),
                out_names=tuple(out_names),
                lowering_input_output_aliases=(),
                sim_require_finite=True,
                sim_require_nnan=True,
                nc=nc,
            )
            return tuple(outs)

        devices = jax.devices()[:n_cores]
        assert len(devices) == n_cores, (
            f"need {n_cores} neuron cores, found {len(jax.devices())}"
        )
        mesh = Mesh(np.asarray(devices), ("core",))
        in_specs = (PartitionSpec("core"),) * (n_params + n_outs)
        out_specs = (PartitionSpec("core"),) * n_outs
        self.sharded = jax.jit(
            shard_map(
                _body,
                mesh=mesh,
                in_specs=in_specs,
                out_specs=out_specs,
                check_rep=False,
            ),
            donate_argnums=donate,
            keep_unused=True,
        )

    def concat_inputs(self, in_maps):
        return [
            np.concatenate(
                [np.asarray(in_maps[c][n]) for c in range(self.n_cores)], axis=0
            )
            for n in self.in_param_names
        ]

    def _zeros(self):
        return [
            np.zeros((self.n_cores * s[0], *s[1:]), d)
            for (s, d) in self.zero_shapes
        ]

    def run_concat(self, concat_in):
        return self.sharded(*concat_in, *self._zeros())

    def run(self, in_maps):
        out_arrs = self.run_concat(self.concat_inputs(in_maps))
        res = []
        for c in range(self.n_cores):
            m = {}
            for i, name in enumerate(self.out_names):
                s = self.out_avals[i].shape
                m[name] = np.asarray(out_arrs[i]).reshape(self.n_cores, *s)[c]
            res.append(m)
        return res


def get_runner(n_iters: int = 1):
    key = ("runner", n_iters)
    if key not in _CACHE:
        nc = _build_program(n_iters=n_iters)
        _CACHE[key] = _SpmdRunner(nc, N_CORES)
    return _CACHE[key]


def make_in_maps(x, W_base, b_base, A, B):
    bf16 = ml_dtypes.bfloat16
    x2d = np.ascontiguousarray(x, dtype=np.float32).reshape(B_ * S, D_IN)
    wT = np.ascontiguousarray(W_base.T.astype(bf16))
    ab = np.ascontiguousarray(
        np.concatenate([A, B.T], axis=1), dtype=np.float32
    )
    bias = np.ascontiguousarray(b_base, dtype=np.float32)
    in_maps = []
    for c in range(N_CORES):
        xc = x2d[c * T_CORE : (c + 1) * T_CORE]  # [2048 t, 2048 k]
        # x4[tt, p(k), kt, t] = xc[tt*128 + t, kt*128 + p]  (SBUF layout)
        x4 = np.ascontiguousarray(
            xc.reshape(N_TT, P, N_KT, P).transpose(0, 3, 2, 1).astype(bf16)
        )
        in_maps.append({"x4": x4, "wT": wT, "ab": ab, "bias": bias})
    return in_maps


def kernel(**inputs):
    x = inputs["x"]
    W_base = inputs["W_base"]
    b_base = inputs["b_base"]
    A = inputs["A"]
    B = inputs["B"]
    runner = get_runner()
    in_maps = make_in_maps(x, W_base, b_base, A, B)
    res = runner.run(in_maps)
    y2d = np.concatenate([res[c]["out"] for c in range(N_CORES)], axis=0)
    return np.ascontiguousarray(y2d.reshape(B_, S, D_OUT), dtype=np.float32)

